# revision 5
# baseline (speedup 1.0000x reference)
"""Locally-connected layer (3x3, stride 1) on 8 Trainium2 NeuronCores.

Shapes (hardcoded):
  x      [B=32, C=96, H=32, W=32]  fp32
  weight [P=900, O=96, K=864]      fp32   (K = C*3*3, channel-major (c,kh,kw))
  bias   [P=900, O=96]             fp32
  out    [B=32, O=96, 30, 30]      fp32

Strategy:
  - Shard the 30x30 patch grid by output rows, padded to 32 rows -> 4 rows
    (120 patches) per core.  One SPMD program on all 8 cores.
  - All operands cast to bf16 on the host: weight DMA halves (the HBM
    roofline term: 19.9 MB/core) and the PE streams 1 col/cycle instead of
    fp32's 1/4 rate.  PSUM accumulation stays fp32; measured rel err ~4e-3
    vs the 2e-2 gate.
  - Per patch, contract K=864 as 9 accumulating matmuls of K=C=96:
    out[b,o] += x[:, i+di, j+dj, b].T @ W[p, dd][:, o].
  - Stationary (lhsT) = x columns [96c, 32b] read in place from an SBUF-resident
    x slice laid out [c, h, w, b]; moving (rhs) = per-patch weight [96c, 96o].
  - Groups of 4 (or 3) adjacent patches are col-tiled onto the 128-wide PE
    array via tile_position=(0, 32u) so their matmuls run concurrently.
  - Weights are streamed from HBM in half-row chunks (15 patches, ~1.24 MB),
    double buffered; host pre-transposes weight to [c, p, dd, o] so each
    chunk is contiguous per partition.
"""

import numpy as np
import ml_dtypes

BF16 = ml_dtypes.bfloat16

B, C, O, H, W = 32, 96, 96, 32, 32
OH = OW = 30
NCORES = 8
ROWS_PER_CORE = 4            # padded 32 output rows / 8 cores
P_CORE = ROWS_PER_CORE * OW  # 120 patches per core
XROWS = ROWS_PER_CORE + 2    # input rows needed per core (halo)
CH = 15                      # patches per weight chunk (half output row)

LAST_RESULT = None           # BassKernelResults of the most recent run
_NC_CACHE = {}
KERNEL_KW = {}               # _build_bass kwargs for the kernel() path


def _chunk_groups(cp):
    """Split a chunk of cp consecutive patches into col-tile groups of <=4."""
    groups, j = [], 0
    while j < cp:
        g = min(4, cp - j)
        if cp - j == 5:      # avoid a trailing group of 1
            g = 3
        groups.append((j, g))
        j += g
    return groups


def _build_bass(reps=1, with_wdma=True, with_mm=True, with_out=True,
                chunk_patches=CH, wbufs=2, out_f32=False, alt_ring=False):
    import concourse.bass as bass
    import concourse.mybir as mybir
    import concourse.tile as tile
    from concourse import bacc

    cp = chunk_patches
    assert OW % cp == 0
    cpr = OW // cp                       # chunks per row
    groups = _chunk_groups(cp)
    n_groups = ROWS_PER_CORE * cpr * len(groups)
    otw = n_groups * O

    f32 = mybir.dt.float32
    bf16 = mybir.dt.bfloat16
    out_dt = f32 if out_f32 else bf16
    nc = bacc.Bacc("TRN2", target_bir_lowering=False, debug=False,
                   num_devices=NCORES)
    xsd = nc.dram_tensor("xs", [C, XROWS, W, B], bf16, kind="ExternalInput")
    wsd = nc.dram_tensor("ws", [C, P_CORE, 9, O], bf16, kind="ExternalInput")
    od = nc.dram_tensor("out", [128, otw], out_dt, kind="ExternalOutput")

    with tile.TileContext(nc) as tc:
        with (
            tc.tile_pool(name="xp", bufs=1) as xp,
            tc.tile_pool(name="wp", bufs=wbufs) as wp,
            tc.tile_pool(name="op", bufs=1) as op,
            tc.tile_pool(name="pp", bufs=8, space=bass.MemorySpace.PSUM) as pp,
        ):
            xt = xp.tile([C, XROWS, W, B], bf16)
            ot = op.tile([128, otw], out_dt)

            wt_fixed = None
            if not with_wdma:
                # mm-only probe: one persistent weight tile, loaded once
                wt_fixed = xp.tile([C, cp, 9, O], bf16)
                nc.sync.dma_start(wt_fixed[:], wsd[:, 0:cp, :, :])
            if not with_mm and with_out:
                nc.vector.memset(ot[:], 0.0)

            for _rep in range(reps):
                nc.gpsimd.dma_start(xt[:], xsd[:])
                for ch in range(ROWS_PER_CORE * cpr):
                    li, ci = ch // cpr, ch % cpr
                    if with_wdma:
                        wt = wp.tile([C, cp, 9, O], bf16)
                        src = wsd[:, ch * cp:(ch + 1) * cp, :, :]
                        if alt_ring and ch % 2 == 1:
                            nc.scalar.dma_start(wt[:], src)
                        else:
                            nc.sync.dma_start(wt[:], src)
                    else:
                        wt = wt_fixed
                    if with_mm:
                        for gi, (jo, gsz) in enumerate(groups):
                            j0 = ci * cp + jo
                            ps = pp.tile([128, O], f32)
                            for dd in range(9):
                                di, dj = dd // 3, dd % 3
                                for u in range(gsz):
                                    nc.tensor.matmul(
                                        ps[32 * u:32 * (u + 1), :],
                                        xt[:, li + di, j0 + u + dj, :],
                                        wt[:, jo + u, dd, :],
                                        start=(dd == 0),
                                        stop=(dd == 8),
                                        tile_position=(0, 32 * u),
                                    )
                            g = (li * cpr + ci) * len(groups) + gi
                            nc.vector.tensor_copy(
                                ot[0:32 * gsz, g * O:(g + 1) * O],
                                ps[0:32 * gsz, :])
                if with_out:
                    nc.sync.dma_start(od[:], ot[:])
    nc.compile()
    return nc


def _build_tiny():
    """Trivial kernel with comparable I/O signature for overhead subtraction."""
    import concourse.mybir as mybir
    import concourse.tile as tile
    from concourse import bacc

    bf16 = mybir.dt.bfloat16
    nc = bacc.Bacc("TRN2", target_bir_lowering=False, debug=False,
                   num_devices=NCORES)
    xsd = nc.dram_tensor("xs", [C, XROWS, W, B], bf16, kind="ExternalInput")
    od = nc.dram_tensor("out", [128, 8], bf16, kind="ExternalOutput")
    with tile.TileContext(nc) as tc:
        with tc.tile_pool(name="tp", bufs=1) as tp:
            t = tp.tile([C, 8], bf16)
            nc.sync.dma_start(t[:], xsd[:, 0, 0:8, 0])
            nc.sync.dma_start(od[0:C, :], t[:])
    nc.compile()
    return nc


def _get_nc():
    key = tuple(sorted(KERNEL_KW.items()))
    if key not in _NC_CACHE:
        _NC_CACHE[key] = _build_bass(**KERNEL_KW)
    return _NC_CACHE[key]


def _prep_in_maps(x, weight):
    # weight [900, O, C*3*3] -> [C, P_pad=960, dd, O] in bf16
    w5 = weight.reshape(OH * OW, O, C, 3, 3)
    wt = w5.transpose(2, 0, 3, 4, 1).reshape(C, OH * OW, 9, O)
    wpad = np.zeros((C, NCORES * P_CORE, 9, O), dtype=BF16)
    wpad[:, :OH * OW] = wt.astype(BF16)

    # x [B, C, H, W] -> [C, H_pad=34, W, B] in bf16
    xt = x.transpose(1, 2, 3, 0)
    xpad = np.zeros((C, H + 2, W, B), dtype=BF16)
    xpad[:, :H] = xt.astype(BF16)

    in_maps = []
    for c in range(NCORES):
        in_maps.append({
            "xs": np.ascontiguousarray(
                xpad[:, ROWS_PER_CORE * c:ROWS_PER_CORE * c + XROWS]),
            "ws": np.ascontiguousarray(
                wpad[:, P_CORE * c:P_CORE * (c + 1)]),
        })
    return in_maps


def kernel(x, weight, bias):
    global LAST_RESULT
    from concourse.bass_utils import run_bass_kernel_spmd

    x = np.asarray(x, dtype=np.float32)
    weight = np.asarray(weight, dtype=np.float32)
    bias = np.asarray(bias, dtype=np.float32)

    in_maps = _prep_in_maps(x, weight)
    nc = _get_nc()
    LAST_RESULT = run_bass_kernel_spmd(
        nc, in_maps, core_ids=list(range(NCORES)), trace=False)

    # ---- gather: per-core [128, n_groups*96] -> full [B, O, 30, 30] ----
    groups = _chunk_groups(CH)
    cpr = OW // CH
    n_groups = ROWS_PER_CORE * cpr * len(groups)
    out = np.zeros((B, O, OH, OW), dtype=np.float32)
    for c in range(NCORES):
        oc = LAST_RESULT.results[c]["out"].astype(np.float32)
        oc = oc.reshape(4, 32, n_groups, O)
        for li in range(ROWS_PER_CORE):
            i = ROWS_PER_CORE * c + li
            if i >= OH:
                continue
            for ci in range(cpr):
                for gi, (jo, gsz) in enumerate(groups):
                    j0 = ci * CH + jo
                    g = (li * cpr + ci) * len(groups) + gi
                    blk = oc[:gsz, :, g, :]            # [u, b, o]
                    out[:, :, i, j0:j0 + gsz] = blk.transpose(1, 2, 0)
    out += bias.reshape(OH, OW, O).transpose(2, 0, 1)[None]
    return out


# revision 7
# speedup vs baseline: 1.0088x; 1.0088x over previous
"""Locally-connected layer (3x3, stride 1) on 8 Trainium2 NeuronCores.

Shapes (hardcoded):
  x      [B=32, C=96, H=32, W=32]  fp32
  weight [P=900, O=96, K=864]      fp32   (K = C*3*3, channel-major (c,kh,kw))
  bias   [P=900, O=96]             fp32
  out    [B=32, O=96, 30, 30]      fp32

Strategy:
  - Shard the 30x30 patch grid by output rows, padded to 32 rows -> 4 rows
    (120 patches) per core.  One SPMD program on all 8 cores.
  - Everything bf16 (PSUM accumulation fp32; rel err ~4e-3 vs 2e-2 gate).
  - The kernel is weight-DMA bound: per-core weight traffic is ~20 MB and
    HBM-per-core peaks at ~355 GB/s ONLY for 128-partition transfers
    (96-partition tiles cap at ~239 GB/s).  So the contraction is repacked
    dd-major to k' = dd*96 + c in [0,864), padded to 7 chunks of 128, and
    weights ship as [128, patch, ki, o] - full-bandwidth DMA.
  - The matching x operand xk[p, ki, li, j, b] = x[c, li+di, j+dj, b]
    (k' = 128*ki+p = dd*96+c, dd=(di,dj)) is built on-device from the tiny
    1.2 MB x tile by 13 SBUF->SBUF DMA segment copies (partition shifts are
    multiples of 32; the DMA crossbar handles them).
  - Per patch: 7 accumulating matmuls K=128 (K=96 for ki=6, skipping the
    zero-pad rows).  Stationary (lhsT) = xk column [128, 32b]; moving =
    per-patch weight [128, 96o].  Groups of 4 (or 3) adjacent patches are
    col-tiled via tile_position=(0, 32u).
  - Weights stream in 8 chunks of 15 patches (2.5 MB), double buffered on
    the sync (HWDGE) ring; x/gathers ride the gpsimd (SWDGE) ring.
"""

import numpy as np
import ml_dtypes

BF16 = ml_dtypes.bfloat16

B, C, O, H, W = 32, 96, 96, 32, 32
OH = OW = 30
NCORES = 8
ROWS_PER_CORE = 4            # padded 32 output rows / 8 cores
P_CORE = ROWS_PER_CORE * OW  # 120 patches per core
XROWS = ROWS_PER_CORE + 2    # input rows needed per core (halo)
CH = 15                      # patches per weight chunk (half output row)
KI = 7                       # contraction chunks: 864 -> 7 x 128 (last 96)

LAST_RESULT = None           # BassKernelResults of the most recent run
_NC_CACHE = {}
KERNEL_KW = {}               # _build_bass kwargs for the kernel() path


def _chunk_groups(cp):
    """Split a chunk of cp consecutive patches into col-tile groups of <=4."""
    groups, j = [], 0
    while j < cp:
        g = min(4, cp - j)
        if cp - j == 5:      # avoid a trailing group of 1
            g = 3
        groups.append((j, g))
        j += g
    return groups


def _k_segments():
    """Segments of the dd-major contraction packing k' = dd*96 + c into
    7 chunks of 128 partitions: (ki, p0, c0, dd, len)."""
    segs = []
    for ki in range(KI):
        p = 0
        while p < 128:
            k = 128 * ki + p
            if k >= 864:
                break
            dd = k // 96
            c0 = k % 96
            ln = min(128 - p, 96 - c0, 864 - k)
            segs.append((ki, p, c0, dd, ln))
            p += ln
    return segs


def _build_bass(reps=1, with_wdma=True, with_mm=True, with_out=True,
                with_gather=True, chunk_patches=CH, wbufs=2, kbufs=2,
                xbufs=2, out_f32=False, alt_ring=False, gather_engine="gpsimd"):
    import concourse.bass as bass
    import concourse.mybir as mybir
    import concourse.tile as tile
    from concourse import bacc

    cp = chunk_patches
    assert OW % cp == 0
    cpr = OW // cp                       # chunks per row
    groups = _chunk_groups(cp)
    n_groups = ROWS_PER_CORE * cpr * len(groups)
    otw = n_groups * O
    segs = _k_segments()

    f32 = mybir.dt.float32
    bf16 = mybir.dt.bfloat16
    out_dt = f32 if out_f32 else bf16
    nc = bacc.Bacc("TRN2", target_bir_lowering=False, debug=False,
                   num_devices=NCORES)
    xsd = nc.dram_tensor("xs", [C, XROWS, W, B], bf16, kind="ExternalInput")
    wsd = nc.dram_tensor("ws", [128, P_CORE, KI, O], bf16, kind="ExternalInput")
    od = nc.dram_tensor("out", [128, otw], out_dt, kind="ExternalOutput")

    with tile.TileContext(nc) as tc:
        with (
            tc.tile_pool(name="xp", bufs=xbufs) as xp,
            tc.tile_pool(name="kp", bufs=kbufs) as kp,
            tc.tile_pool(name="wp", bufs=wbufs) as wp,
            tc.tile_pool(name="op", bufs=1) as op,
            tc.tile_pool(name="pp", bufs=8, space=bass.MemorySpace.PSUM) as pp,
        ):
            ot = op.tile([128, otw], out_dt)
            if not with_mm and with_out:
                nc.vector.memset(ot[:], 0.0)

            wt_fixed = None
            if not with_wdma:
                wt_fixed = wp.tile([128, cp, KI, O], bf16)
                nc.sync.dma_start(wt_fixed[:], wsd[:, 0:cp, :, :])

            for _rep in range(reps):
                xt = xp.tile([C, XROWS, W, B], bf16)
                nc.gpsimd.dma_start(xt[:], xsd[:])
                xk = kp.tile([128, KI, ROWS_PER_CORE, OW, B], bf16)
                if with_gather:
                    eng = {"gpsimd": nc.gpsimd.dma_start,
                           "vector": nc.vector.tensor_copy,
                           "scalar": nc.scalar.copy,
                           "gpsimd_tc": nc.gpsimd.tensor_copy}[gather_engine]
                    for (ki, p0, c0, dd, ln) in segs:
                        di, dj = dd // 3, dd % 3
                        eng(
                            xk[p0:p0 + ln, ki, :, :, :],
                            xt[c0:c0 + ln, di:di + ROWS_PER_CORE,
                               dj:dj + OW, :])

                for ch in range(ROWS_PER_CORE * cpr):
                    li, ci = ch // cpr, ch % cpr
                    if with_wdma:
                        wt = wp.tile([128, cp, KI, O], bf16)
                        src = wsd[:, ch * cp:(ch + 1) * cp, :, :]
                        if alt_ring and ch % 2 == 1:
                            nc.scalar.dma_start(wt[:], src)
                        else:
                            nc.sync.dma_start(wt[:], src)
                    else:
                        wt = wt_fixed
                    if with_mm:
                        for gi, (jo, gsz) in enumerate(groups):
                            j0 = ci * cp + jo
                            ps = pp.tile([128, O], f32)
                            for ki in range(KI):
                                kk = 96 if ki == KI - 1 else 128
                                for u in range(gsz):
                                    nc.tensor.matmul(
                                        ps[32 * u:32 * (u + 1), :],
                                        xk[0:kk, ki, li, j0 + u, :],
                                        wt[0:kk, jo + u, ki, :],
                                        start=(ki == 0),
                                        stop=(ki == KI - 1),
                                        tile_position=(0, 32 * u),
                                    )
                            g = (li * cpr + ci) * len(groups) + gi
                            nc.vector.tensor_copy(
                                ot[0:32 * gsz, g * O:(g + 1) * O],
                                ps[0:32 * gsz, :])
                if with_out:
                    nc.sync.dma_start(od[:], ot[:])
    nc.compile()
    return nc


def _build_tiny():
    """Trivial kernel with comparable I/O signature for overhead subtraction."""
    import concourse.mybir as mybir
    import concourse.tile as tile
    from concourse import bacc

    bf16 = mybir.dt.bfloat16
    nc = bacc.Bacc("TRN2", target_bir_lowering=False, debug=False,
                   num_devices=NCORES)
    xsd = nc.dram_tensor("xs", [C, XROWS, W, B], bf16, kind="ExternalInput")
    od = nc.dram_tensor("out", [128, 8], bf16, kind="ExternalOutput")
    with tile.TileContext(nc) as tc:
        with tc.tile_pool(name="tp", bufs=1) as tp:
            t = tp.tile([C, 8], bf16)
            nc.sync.dma_start(t[:], xsd[:, 0, 0:8, 0])
            nc.sync.dma_start(od[0:C, :], t[:])
    nc.compile()
    return nc


def _get_nc():
    key = tuple(sorted(KERNEL_KW.items()))
    if key not in _NC_CACHE:
        _NC_CACHE[key] = _build_bass(**KERNEL_KW)
    return _NC_CACHE[key]


def _prep_in_maps(x, weight):
    # weight [900, O, 864] (k = c*9+dd) -> dd-major k' = dd*96+c, padded to
    # 896 = 7*128, laid out [128, P_pad=960, ki, O] in bf16
    wddc = weight.reshape(OH * OW, O, C, 9).transpose(3, 2, 0, 1)  # [9,C,900,O]
    wfull = np.zeros((KI * 128, NCORES * P_CORE, O), dtype=BF16)
    wfull[:864, :OH * OW] = wddc.reshape(864, OH * OW, O).astype(BF16)
    wkk = wfull.reshape(KI, 128, NCORES * P_CORE, O).transpose(1, 2, 0, 3)

    # x [B, C, H, W] -> [C, H_pad=34, W, B] in bf16
    xt = x.transpose(1, 2, 3, 0)
    xpad = np.zeros((C, H + 2, W, B), dtype=BF16)
    xpad[:, :H] = xt.astype(BF16)

    in_maps = []
    for c in range(NCORES):
        in_maps.append({
            "xs": np.ascontiguousarray(
                xpad[:, ROWS_PER_CORE * c:ROWS_PER_CORE * c + XROWS]),
            "ws": np.ascontiguousarray(
                wkk[:, P_CORE * c:P_CORE * (c + 1)]),
        })
    return in_maps


def kernel(x, weight, bias):
    global LAST_RESULT
    from concourse.bass_utils import run_bass_kernel_spmd

    x = np.asarray(x, dtype=np.float32)
    weight = np.asarray(weight, dtype=np.float32)
    bias = np.asarray(bias, dtype=np.float32)

    in_maps = _prep_in_maps(x, weight)
    nc = _get_nc()
    LAST_RESULT = run_bass_kernel_spmd(
        nc, in_maps, core_ids=list(range(NCORES)), trace=False)

    # ---- gather: per-core [128, n_groups*96] -> full [B, O, 30, 30] ----
    groups = _chunk_groups(CH)
    cpr = OW // CH
    n_groups = ROWS_PER_CORE * cpr * len(groups)
    out = np.zeros((B, O, OH, OW), dtype=np.float32)
    for c in range(NCORES):
        oc = LAST_RESULT.results[c]["out"].astype(np.float32)
        oc = oc.reshape(4, 32, n_groups, O)
        for li in range(ROWS_PER_CORE):
            i = ROWS_PER_CORE * c + li
            if i >= OH:
                continue
            for ci in range(cpr):
                for gi, (jo, gsz) in enumerate(groups):
                    j0 = ci * CH + jo
                    g = (li * cpr + ci) * len(groups) + gi
                    blk = oc[:gsz, :, g, :]            # [u, b, o]
                    out[:, :, i, j0:j0 + gsz] = blk.transpose(1, 2, 0)
    out += bias.reshape(OH, OW, O).transpose(2, 0, 1)[None]
    return out


# revision 11
# speedup vs baseline: 1.5942x; 1.5803x over previous
"""Locally-connected layer (3x3, stride 1) on 8 Trainium2 NeuronCores.

Shapes (hardcoded):
  x      [B=32, C=96, H=32, W=32]  fp32
  weight [P=900, O=96, K=864]      fp32   (K = C*3*3, channel-major (c,kh,kw))
  bias   [P=900, O=96]             fp32
  out    [B=32, O=96, 30, 30]      fp32

Strategy:
  - Shard the 30x30 patch grid by output rows, padded to 32 rows -> 4 rows
    (120 patches) per core.  One SPMD program on all 8 cores.
  - Everything bf16 (PSUM accumulation fp32; rel err ~4e-3 vs 2e-2 gate).
  - The kernel is weight-DMA bound: per-core weight traffic is ~20 MB and
    HBM-per-core peaks at ~355 GB/s ONLY for 128-partition transfers
    (96-partition tiles cap at ~239 GB/s).  So the contraction is repacked
    dd-major to k' = dd*96 + c in [0,864), padded to 7 chunks of 128, and
    weights ship as [128, patch, ki, o] - full-bandwidth DMA.
  - The matching x operand xk[p, ki, li, j, b] = x[c, li+di, j+dj, b]
    (k' = 128*ki+p = dd*96+c, dd=(di,dj)) is built on-device from the tiny
    1.2 MB x tile by 13 SBUF->SBUF DMA segment copies (partition shifts are
    multiples of 32; the DMA crossbar handles them).
  - Per patch: 7 accumulating matmuls K=128 (K=96 for ki=6, skipping the
    zero-pad rows).  Stationary (lhsT) = xk column [128, 32b]; moving =
    per-patch weight [128, 96o].  Groups of 4 (or 3) adjacent patches are
    col-tiled via tile_position=(0, 32u).
  - Weights stream in 8 chunks of 15 patches (2.5 MB), double buffered on
    the sync (HWDGE) ring; x/gathers ride the gpsimd (SWDGE) ring.
"""

import numpy as np
import ml_dtypes

BF16 = ml_dtypes.bfloat16

B, C, O, H, W = 32, 96, 96, 32, 32
OH = OW = 30
NCORES = 8
ROWS_PER_CORE = 4            # padded 32 output rows / 8 cores
P_CORE = ROWS_PER_CORE * OW  # 120 patches per core
XROWS = ROWS_PER_CORE + 2    # input rows needed per core (halo)
CH = 15                      # patches per weight chunk (half output row)
KI = 7                       # contraction chunks: 864 -> 7 x 128 (last 96)

LAST_RESULT = None           # BassKernelResults of the most recent run
_NC_CACHE = {}
KERNEL_KW = {}               # _build_bass kwargs for the kernel() path


def _chunk_groups(cp):
    """Split a chunk of cp consecutive patches into col-tile groups of <=4."""
    groups, j = [], 0
    while j < cp:
        g = min(4, cp - j)
        if cp - j == 5:      # avoid a trailing group of 1
            g = 3
        groups.append((j, g))
        j += g
    return groups


def _k_segments():
    """Segments of the dd-major contraction packing k' = dd*96 + c into
    7 chunks of 128 partitions: (ki, p0, c0, dd, len)."""
    segs = []
    for ki in range(KI):
        p = 0
        while p < 128:
            k = 128 * ki + p
            if k >= 864:
                break
            dd = k // 96
            c0 = k % 96
            ln = min(128 - p, 96 - c0, 864 - k)
            segs.append((ki, p, c0, dd, ln))
            p += ln
    return segs


def _build_bass(reps=1, with_wdma=True, with_mm=True, with_out=True,
                with_gather=True, chunk_patches=CH, wbufs=3, kbufs=2,
                xbufs=2, out_f32=False, alt_ring=False, gather_engine="vector",
                row_out=True):
    import concourse.bass as bass
    import concourse.mybir as mybir
    import concourse.tile as tile
    from concourse import bacc

    cp = chunk_patches
    assert OW % cp == 0
    cpr = OW // cp                       # chunks per row
    groups = _chunk_groups(cp)
    n_groups = ROWS_PER_CORE * cpr * len(groups)
    otw = n_groups * O
    segs = _k_segments()

    f32 = mybir.dt.float32
    bf16 = mybir.dt.bfloat16
    out_dt = f32 if out_f32 else bf16
    nc = bacc.Bacc("TRN2", target_bir_lowering=False, debug=False,
                   num_devices=NCORES)
    xsd = nc.dram_tensor("xs", [C, XROWS, W, B], bf16, kind="ExternalInput")
    wsd = nc.dram_tensor("ws", [128, P_CORE, KI, O], bf16, kind="ExternalInput")
    od = nc.dram_tensor("out", [128, otw], out_dt, kind="ExternalOutput")

    with tile.TileContext(nc) as tc:
        with (
            tc.tile_pool(name="xp", bufs=xbufs) as xp,
            tc.tile_pool(name="kp", bufs=kbufs) as kp,
            tc.tile_pool(name="wp", bufs=wbufs) as wp,
            tc.tile_pool(name="op", bufs=1) as op,
            tc.tile_pool(name="pp", bufs=8, space=bass.MemorySpace.PSUM) as pp,
        ):
            ot = op.tile([128, otw], out_dt)
            if not with_mm and with_out:
                nc.vector.memset(ot[:], 0.0)

            wt_fixed = None
            if not with_wdma:
                wt_fixed = wp.tile([128, cp, KI, O], bf16)
                nc.sync.dma_start(wt_fixed[:], wsd[:, 0:cp, :, :])

            for _rep in range(reps):
                xt = xp.tile([C, XROWS, W, B], bf16)
                nc.gpsimd.dma_start(xt[:], xsd[:])
                xk = kp.tile([128, KI, ROWS_PER_CORE, OW, B], bf16)
                if with_gather:
                    eng = {"gpsimd": nc.gpsimd.dma_start,
                           "scalar_dma": nc.scalar.dma_start,
                           "sync_dma": nc.sync.dma_start,
                           "vector": nc.vector.tensor_copy,
                           "scalar": nc.scalar.copy,
                           "gpsimd_tc": nc.gpsimd.tensor_copy}[gather_engine]
                    # DVE/ACT engine copies: <=32 partitions per op when the
                    # base partition is non-zero, so split into 32-blocks.
                    blk = 32 if gather_engine in (
                        "vector", "scalar", "gpsimd_tc") else 128
                    for (ki, p0, c0, dd, ln) in segs:
                        di, dj = dd // 3, dd % 3
                        step = ln if (p0 == 0 and c0 == 0) else min(blk, ln)
                        for q in range(0, ln, step):
                            eng(
                                xk[p0 + q:p0 + q + step, ki, :, :, :],
                                xt[c0 + q:c0 + q + step,
                                   di:di + ROWS_PER_CORE, dj:dj + OW, :])

                for ch in range(ROWS_PER_CORE * cpr):
                    li, ci = ch // cpr, ch % cpr
                    if with_wdma:
                        wt = wp.tile([128, cp, KI, O], bf16)
                        src = wsd[:, ch * cp:(ch + 1) * cp, :, :]
                        if alt_ring and ch % 2 == 1:
                            nc.scalar.dma_start(wt[:], src)
                        else:
                            nc.sync.dma_start(wt[:], src)
                    else:
                        wt = wt_fixed
                    if with_mm:
                        for gi, (jo, gsz) in enumerate(groups):
                            j0 = ci * cp + jo
                            ps = pp.tile([128, O], f32)
                            for ki in range(KI):
                                kk = 96 if ki == KI - 1 else 128
                                for u in range(gsz):
                                    nc.tensor.matmul(
                                        ps[32 * u:32 * (u + 1), :],
                                        xk[0:kk, ki, li, j0 + u, :],
                                        wt[0:kk, jo + u, ki, :],
                                        start=(ki == 0),
                                        stop=(ki == KI - 1),
                                        tile_position=(0, 32 * u),
                                    )
                            g = (li * cpr + ci) * len(groups) + gi
                            nc.vector.tensor_copy(
                                ot[0:32 * gsz, g * O:(g + 1) * O],
                                ps[0:32 * gsz, :])
                    if with_out and row_out and ci == cpr - 1:
                        gw = cpr * len(groups) * O
                        nc.gpsimd.dma_start(od[:, li * gw:(li + 1) * gw],
                                            ot[:, li * gw:(li + 1) * gw])
                if with_out and not row_out:
                    nc.sync.dma_start(od[:], ot[:])
    nc.compile()
    return nc


def _build_tiny():
    """Trivial kernel with comparable I/O signature for overhead subtraction."""
    import concourse.mybir as mybir
    import concourse.tile as tile
    from concourse import bacc

    bf16 = mybir.dt.bfloat16
    nc = bacc.Bacc("TRN2", target_bir_lowering=False, debug=False,
                   num_devices=NCORES)
    xsd = nc.dram_tensor("xs", [C, XROWS, W, B], bf16, kind="ExternalInput")
    od = nc.dram_tensor("out", [128, 8], bf16, kind="ExternalOutput")
    with tile.TileContext(nc) as tc:
        with tc.tile_pool(name="tp", bufs=1) as tp:
            t = tp.tile([C, 8], bf16)
            nc.sync.dma_start(t[:], xsd[:, 0, 0:8, 0])
            nc.sync.dma_start(od[0:C, :], t[:])
    nc.compile()
    return nc


def _get_nc():
    key = tuple(sorted(KERNEL_KW.items()))
    if key not in _NC_CACHE:
        _NC_CACHE[key] = _build_bass(**KERNEL_KW)
    return _NC_CACHE[key]


def _prep_in_maps(x, weight):
    # weight [900, O, 864] (k = c*9+dd) -> dd-major k' = dd*96+c, padded to
    # 896 = 7*128, laid out [128, P_pad=960, ki, O] in bf16
    wddc = weight.reshape(OH * OW, O, C, 9).transpose(3, 2, 0, 1)  # [9,C,900,O]
    wfull = np.zeros((KI * 128, NCORES * P_CORE, O), dtype=BF16)
    wfull[:864, :OH * OW] = wddc.reshape(864, OH * OW, O).astype(BF16)
    wkk = wfull.reshape(KI, 128, NCORES * P_CORE, O).transpose(1, 2, 0, 3)

    # x [B, C, H, W] -> [C, H_pad=34, W, B] in bf16
    xt = x.transpose(1, 2, 3, 0)
    xpad = np.zeros((C, H + 2, W, B), dtype=BF16)
    xpad[:, :H] = xt.astype(BF16)

    in_maps = []
    for c in range(NCORES):
        in_maps.append({
            "xs": np.ascontiguousarray(
                xpad[:, ROWS_PER_CORE * c:ROWS_PER_CORE * c + XROWS]),
            "ws": np.ascontiguousarray(
                wkk[:, P_CORE * c:P_CORE * (c + 1)]),
        })
    return in_maps


def kernel(x, weight, bias):
    global LAST_RESULT
    from concourse.bass_utils import run_bass_kernel_spmd

    x = np.asarray(x, dtype=np.float32)
    weight = np.asarray(weight, dtype=np.float32)
    bias = np.asarray(bias, dtype=np.float32)

    in_maps = _prep_in_maps(x, weight)
    nc = _get_nc()
    LAST_RESULT = run_bass_kernel_spmd(
        nc, in_maps, core_ids=list(range(NCORES)), trace=False)

    # ---- gather: per-core [128, n_groups*96] -> full [B, O, 30, 30] ----
    groups = _chunk_groups(CH)
    cpr = OW // CH
    n_groups = ROWS_PER_CORE * cpr * len(groups)
    out = np.zeros((B, O, OH, OW), dtype=np.float32)
    for c in range(NCORES):
        oc = LAST_RESULT.results[c]["out"].astype(np.float32)
        oc = oc.reshape(4, 32, n_groups, O)
        for li in range(ROWS_PER_CORE):
            i = ROWS_PER_CORE * c + li
            if i >= OH:
                continue
            for ci in range(cpr):
                for gi, (jo, gsz) in enumerate(groups):
                    j0 = ci * CH + jo
                    g = (li * cpr + ci) * len(groups) + gi
                    blk = oc[:gsz, :, g, :]            # [u, b, o]
                    out[:, :, i, j0:j0 + gsz] = blk.transpose(1, 2, 0)
    out += bias.reshape(OH, OW, O).transpose(2, 0, 1)[None]
    return out


# revision 12
# speedup vs baseline: 1.9623x; 1.2309x over previous
"""Locally-connected layer (3x3, stride 1) on 8 Trainium2 NeuronCores.

Shapes (hardcoded):
  x      [B=32, C=96, H=32, W=32]  fp32
  weight [P=900, O=96, K=864]      fp32   (K = C*3*3, channel-major (c,kh,kw))
  bias   [P=900, O=96]             fp32
  out    [B=32, O=96, 30, 30]      fp32

Strategy:
  - Shard the 30x30 patch grid by output rows, padded to 32 rows -> 4 rows
    (120 patches) per core.  One SPMD program on all 8 cores.
  - Everything bf16 (PSUM accumulation fp32; rel err ~4e-3 vs 2e-2 gate).
  - The kernel is weight-DMA bound: per-core weight traffic is ~20 MB and
    HBM-per-core peaks at ~355 GB/s ONLY for 128-partition transfers
    (96-partition tiles cap at ~239 GB/s).  So the contraction is repacked
    dd-major to k' = dd*96 + c in [0,864), padded to 7 chunks of 128, and
    weights ship as [128, patch, ki, o] - full-bandwidth DMA.
  - The matching x operand xk[p, ki, li, j, b] = x[c, li+di, j+dj, b]
    (k' = 128*ki+p = dd*96+c, dd=(di,dj)) is built on-device from the tiny
    1.2 MB x tile by DVE tensor_copy segment copies.  The partition shifts
    are multiples of 32; DVE accepts shifted copies of <=32 partitions at a
    time (walrus verifier rule), so shifted segments split into 32-blocks.
    Keeping the gather OFF the DMA fabric is worth ~50us: SBUF->SBUF DMA
    gathers contend with the weight stream for the 16 SDMA AXI ports.
  - Per patch: 7 accumulating matmuls K=128 (K=96 for ki=6, skipping the
    zero-pad rows).  Stationary (lhsT) = xk column [128, 32b]; moving =
    per-patch weight [128, 96o].  Groups of 4 (or 3) adjacent patches are
    col-tiled via tile_position=(0, 32u).  Tensor time ~35us, hidden.
  - Weights stream in 8 chunks of 15 patches (2.5 MB), triple buffered on
    the sync (HWDGE) ring (wbufs=3 keeps the SDMA queue saturated: ~420
    vs ~355 GB/s/core); x and the per-row output DMAs ride the gpsimd
    (SWDGE) ring.  Measured ~49-57us fresh / ~70us under sustained load,
    vs the ~44us aggregate-HBM floor for the 149 MB bf16 weight stream.
"""

import numpy as np
import ml_dtypes

BF16 = ml_dtypes.bfloat16

B, C, O, H, W = 32, 96, 96, 32, 32
OH = OW = 30
NCORES = 8
ROWS_PER_CORE = 4            # padded 32 output rows / 8 cores
P_CORE = ROWS_PER_CORE * OW  # 120 patches per core
XROWS = ROWS_PER_CORE + 2    # input rows needed per core (halo)
CH = 15                      # patches per weight chunk (half output row)
KI = 7                       # contraction chunks: 864 -> 7 x 128 (last 96)

LAST_RESULT = None           # BassKernelResults of the most recent run
_NC_CACHE = {}
KERNEL_KW = {}               # _build_bass kwargs for the kernel() path


def _chunk_groups(cp):
    """Split a chunk of cp consecutive patches into col-tile groups of <=4."""
    groups, j = [], 0
    while j < cp:
        g = min(4, cp - j)
        if cp - j == 5:      # avoid a trailing group of 1
            g = 3
        groups.append((j, g))
        j += g
    return groups


def _k_segments():
    """Segments of the dd-major contraction packing k' = dd*96 + c into
    7 chunks of 128 partitions: (ki, p0, c0, dd, len)."""
    segs = []
    for ki in range(KI):
        p = 0
        while p < 128:
            k = 128 * ki + p
            if k >= 864:
                break
            dd = k // 96
            c0 = k % 96
            ln = min(128 - p, 96 - c0, 864 - k)
            segs.append((ki, p, c0, dd, ln))
            p += ln
    return segs


def _build_bass(reps=1, with_wdma=True, with_mm=True, with_out=True,
                with_gather=True, chunk_patches=CH, wbufs=3, kbufs=2,
                xbufs=2, out_f32=False, alt_ring=False, gather_engine="vector",
                row_out=True):
    import concourse.bass as bass
    import concourse.mybir as mybir
    import concourse.tile as tile
    from concourse import bacc

    cp = chunk_patches
    assert OW % cp == 0
    cpr = OW // cp                       # chunks per row
    groups = _chunk_groups(cp)
    n_groups = ROWS_PER_CORE * cpr * len(groups)
    otw = n_groups * O
    segs = _k_segments()

    f32 = mybir.dt.float32
    bf16 = mybir.dt.bfloat16
    out_dt = f32 if out_f32 else bf16
    nc = bacc.Bacc("TRN2", target_bir_lowering=False, debug=False,
                   num_devices=NCORES)
    xsd = nc.dram_tensor("xs", [C, XROWS, W, B], bf16, kind="ExternalInput")
    wsd = nc.dram_tensor("ws", [128, P_CORE, KI, O], bf16, kind="ExternalInput")
    od = nc.dram_tensor("out", [128, otw], out_dt, kind="ExternalOutput")

    with tile.TileContext(nc) as tc:
        with (
            tc.tile_pool(name="xp", bufs=xbufs) as xp,
            tc.tile_pool(name="kp", bufs=kbufs) as kp,
            tc.tile_pool(name="wp", bufs=wbufs) as wp,
            tc.tile_pool(name="op", bufs=1) as op,
            tc.tile_pool(name="pp", bufs=8, space=bass.MemorySpace.PSUM) as pp,
        ):
            ot = op.tile([128, otw], out_dt)
            if not with_mm and with_out:
                nc.vector.memset(ot[:], 0.0)

            wt_fixed = None
            if not with_wdma:
                wt_fixed = wp.tile([128, cp, KI, O], bf16)
                nc.sync.dma_start(wt_fixed[:], wsd[:, 0:cp, :, :])

            for _rep in range(reps):
                xt = xp.tile([C, XROWS, W, B], bf16)
                nc.gpsimd.dma_start(xt[:], xsd[:])
                xk = kp.tile([128, KI, ROWS_PER_CORE, OW, B], bf16)
                if with_gather:
                    eng = {"gpsimd": nc.gpsimd.dma_start,
                           "scalar_dma": nc.scalar.dma_start,
                           "sync_dma": nc.sync.dma_start,
                           "vector": nc.vector.tensor_copy,
                           "scalar": nc.scalar.copy,
                           "gpsimd_tc": nc.gpsimd.tensor_copy}[gather_engine]
                    # DVE/ACT engine copies: <=32 partitions per op when the
                    # base partition is non-zero, so split into 32-blocks.
                    blk = 32 if gather_engine in (
                        "vector", "scalar", "gpsimd_tc") else 128
                    for (ki, p0, c0, dd, ln) in segs:
                        di, dj = dd // 3, dd % 3
                        step = ln if (p0 == 0 and c0 == 0) else min(blk, ln)
                        for q in range(0, ln, step):
                            eng(
                                xk[p0 + q:p0 + q + step, ki, :, :, :],
                                xt[c0 + q:c0 + q + step,
                                   di:di + ROWS_PER_CORE, dj:dj + OW, :])

                for ch in range(ROWS_PER_CORE * cpr):
                    li, ci = ch // cpr, ch % cpr
                    if with_wdma:
                        wt = wp.tile([128, cp, KI, O], bf16)
                        src = wsd[:, ch * cp:(ch + 1) * cp, :, :]
                        if alt_ring and ch % 2 == 1:
                            nc.scalar.dma_start(wt[:], src)
                        else:
                            nc.sync.dma_start(wt[:], src)
                    else:
                        wt = wt_fixed
                    if with_mm:
                        for gi, (jo, gsz) in enumerate(groups):
                            j0 = ci * cp + jo
                            ps = pp.tile([128, O], f32)
                            for ki in range(KI):
                                kk = 96 if ki == KI - 1 else 128
                                for u in range(gsz):
                                    nc.tensor.matmul(
                                        ps[32 * u:32 * (u + 1), :],
                                        xk[0:kk, ki, li, j0 + u, :],
                                        wt[0:kk, jo + u, ki, :],
                                        start=(ki == 0),
                                        stop=(ki == KI - 1),
                                        tile_position=(0, 32 * u),
                                    )
                            g = (li * cpr + ci) * len(groups) + gi
                            nc.vector.tensor_copy(
                                ot[0:32 * gsz, g * O:(g + 1) * O],
                                ps[0:32 * gsz, :])
                    if with_out and row_out and ci == cpr - 1:
                        gw = cpr * len(groups) * O
                        nc.gpsimd.dma_start(od[:, li * gw:(li + 1) * gw],
                                            ot[:, li * gw:(li + 1) * gw])
                if with_out and not row_out:
                    nc.sync.dma_start(od[:], ot[:])
    nc.compile()
    return nc


def _build_tiny():
    """Trivial kernel with comparable I/O signature for overhead subtraction."""
    import concourse.mybir as mybir
    import concourse.tile as tile
    from concourse import bacc

    bf16 = mybir.dt.bfloat16
    nc = bacc.Bacc("TRN2", target_bir_lowering=False, debug=False,
                   num_devices=NCORES)
    xsd = nc.dram_tensor("xs", [C, XROWS, W, B], bf16, kind="ExternalInput")
    od = nc.dram_tensor("out", [128, 8], bf16, kind="ExternalOutput")
    with tile.TileContext(nc) as tc:
        with tc.tile_pool(name="tp", bufs=1) as tp:
            t = tp.tile([C, 8], bf16)
            nc.sync.dma_start(t[:], xsd[:, 0, 0:8, 0])
            nc.sync.dma_start(od[0:C, :], t[:])
    nc.compile()
    return nc


def _get_nc():
    key = tuple(sorted(KERNEL_KW.items()))
    if key not in _NC_CACHE:
        _NC_CACHE[key] = _build_bass(**KERNEL_KW)
    return _NC_CACHE[key]


def _prep_in_maps(x, weight):
    # weight [900, O, 864] (k = c*9+dd) -> dd-major k' = dd*96+c, padded to
    # 896 = 7*128, laid out [128, P_pad=960, ki, O] in bf16
    wddc = weight.reshape(OH * OW, O, C, 9).transpose(3, 2, 0, 1)  # [9,C,900,O]
    wfull = np.zeros((KI * 128, NCORES * P_CORE, O), dtype=BF16)
    wfull[:864, :OH * OW] = wddc.reshape(864, OH * OW, O).astype(BF16)
    wkk = wfull.reshape(KI, 128, NCORES * P_CORE, O).transpose(1, 2, 0, 3)

    # x [B, C, H, W] -> [C, H_pad=34, W, B] in bf16
    xt = x.transpose(1, 2, 3, 0)
    xpad = np.zeros((C, H + 2, W, B), dtype=BF16)
    xpad[:, :H] = xt.astype(BF16)

    in_maps = []
    for c in range(NCORES):
        in_maps.append({
            "xs": np.ascontiguousarray(
                xpad[:, ROWS_PER_CORE * c:ROWS_PER_CORE * c + XROWS]),
            "ws": np.ascontiguousarray(
                wkk[:, P_CORE * c:P_CORE * (c + 1)]),
        })
    return in_maps


def kernel(x, weight, bias):
    global LAST_RESULT
    from concourse.bass_utils import run_bass_kernel_spmd

    x = np.asarray(x, dtype=np.float32)
    weight = np.asarray(weight, dtype=np.float32)
    bias = np.asarray(bias, dtype=np.float32)

    in_maps = _prep_in_maps(x, weight)
    nc = _get_nc()
    LAST_RESULT = run_bass_kernel_spmd(
        nc, in_maps, core_ids=list(range(NCORES)), trace=False)

    # ---- gather: per-core [128, n_groups*96] -> full [B, O, 30, 30] ----
    groups = _chunk_groups(CH)
    cpr = OW // CH
    n_groups = ROWS_PER_CORE * cpr * len(groups)
    out = np.zeros((B, O, OH, OW), dtype=np.float32)
    for c in range(NCORES):
        oc = LAST_RESULT.results[c]["out"].astype(np.float32)
        oc = oc.reshape(4, 32, n_groups, O)
        for li in range(ROWS_PER_CORE):
            i = ROWS_PER_CORE * c + li
            if i >= OH:
                continue
            for ci in range(cpr):
                for gi, (jo, gsz) in enumerate(groups):
                    j0 = ci * CH + jo
                    g = (li * cpr + ci) * len(groups) + gi
                    blk = oc[:gsz, :, g, :]            # [u, b, o]
                    out[:, :, i, j0:j0 + gsz] = blk.transpose(1, 2, 0)
    out += bias.reshape(OH, OW, O).transpose(2, 0, 1)[None]
    return out
